# revision 27
# baseline (speedup 1.0000x reference)
"""MoE-LoRA double GEMM on 8 Trainium2 NeuronCores (fp16 I/O).

Computes, for E=4 experts:  h_e = x @ A_e^T ; y_e = h_e @ B_e^T
with x:[4,2048,4096] f32, A:[4,64,4096], B:[4,4096,64] ->
y:[4,4,2048,4096] f32.

Strategy: data-parallel shard x over tokens (8192 tokens -> 1024/core),
replicate the small expert weights. All device I/O is fp16 (the extra
rounding is ~4e-4 rel err, far under the 2e-2 gate), which halves HBM
traffic vs fp32 (~46 MB/core) and doubles TensorE throughput; PSUM
accumulation stays fp32. Host prepares matmul-native layouts (x^T with
the contraction dim D leading, A/B transposed + expert-pair packed).
x is split into a small "head" section (first 256 tokens; 8 small DMAs
so GEMM1 of tile 0 finishes early and the store stream starts ~18us in)
and a "body" section (768 tokens; 8 large 768 KiB DMAs for DMA
efficiency), giving pipeline tiles of [256, 256, 512] tokens.
  GEMM1: h^T[pair] = [A_2p^T | A_2p+1^T] (stationary, experts packed on
         the M axis) x x^T tile (moving) accumulated over D.
  GEMM2: y_e tile [128 tok, 512 out] = h_e^T chunk (stationary, K=64,
         the two experts of a pair on row strips 0/64 so their matmuls
         run concurrently in the PE array) x B_e^T (moving), giving y
         in natural [token, out] layout; a full O=4096 row per 128
         tokens is staged in SBUF as fp16 and stored as one 1 MiB DMA.
GEMM1 matmuls of tile t+1 run as dense blocks at GEMM2 group boundaries
(not finer) so LDWEIGHTS stays pipelined: uniform streams keep the
foreground/background weight buffers alternating, while mixing a third
stationary between GEMM2's two row-strip stationaries serializes the
weight loads. PSUM->SBUF evacuation (with the f32->f16 cast) is split
VectorE/ScalarE per output tile. The kernel is HBM/DMA bound.
"""

import os
import sys

import numpy as np

for _p in ("/opt/trn_rl_repo", "/root/.axon_site/_ro/trn_rl_repo"):
    if os.path.isdir(_p) and _p not in sys.path:
        sys.path.append(_p)

from concourse import bacc, mybir, tile
from concourse.bass_utils import run_bass_kernel_spmd

E = 4
R_E = 64
D = 4096
O = 4096
B_DIM = 4
S = 2048
T = B_DIM * S          # 8192 tokens total
NCORES = 8
TL = T // NCORES       # 1024 tokens per core
TTS = [256, 256, 512]  # pipeline tile sizes (tokens); sum == TL
T_HEAD = TTS[0]        # x "head" section: loaded in small per-group DMAs
T_BODY = TL - T_HEAD   # x "body" section: loaded in 8 large DMAs
NT = len(TTS)
NCD = D // 128         # 32 contraction chunks for GEMM1
NG4 = NCD // 4         # 8 groups of 4 chunks (one x DMA each)

F16 = mybir.dt.float16
FP32 = mybir.dt.float32

_CACHE = {}


def _build_nc():
    nc = bacc.Bacc(None, target_bir_lowering=False, debug=False)
    x_d = [
        nc.declare_dram_parameter("x0", [128, NG4, 4, TTS[0]], F16, isOutput=False),
        nc.declare_dram_parameter("x1", [128, NG4, 4, TTS[1]], F16, isOutput=False),
        nc.declare_dram_parameter("x2", [2, 128, NG4 // 2, 4, TTS[2]], F16, isOutput=False),
    ]
    at_d = nc.declare_dram_parameter("AT", [128, 2, NCD, 128], F16, isOutput=False)
    bt_d = nc.declare_dram_parameter("BT", [128, 2, O], F16, isOutput=False)
    y_d = nc.declare_dram_parameter("y", [E, TL, O], F16, isOutput=True)

    with tile.TileContext(nc) as tc:
        with (
            tc.tile_pool(name="ps_y", bufs=3, space="PSUM") as ps_y,
            tc.tile_pool(name="ps_ht", bufs=2, space="PSUM") as ps_ht,
            tc.tile_pool(name="atc", bufs=2) as atpool,
            tc.tile_pool(name="btc", bufs=2) as btpool,
            tc.tile_pool(name="xt", bufs=1) as xtpool,
            tc.tile_pool(name="ht", bufs=2 * NT) as htpool,
            tc.tile_pool(name="ys", bufs=5) as yspool,
        ):
            # ---- loads (ScalarE HWDGE ring; stores go on the SyncE ring) ----
            # 6 large DMAs total: Tile has only 8 DMA-completion sem lanes
            # shared by all engines, and a dispatch whose lane is still held
            # by an earlier in-flight DMA blocks its engine's whole FIFO --
            # many small loads delay both the copies behind them on ScalarE
            # and the first stores on SyncE. Order: A, x tile0, B, x1, x2
            # so each tile's x lands before the GEMM2 window it overlaps.
            atc = atpool.tile([128, 2, NCD, 128], F16, name="atc", tag="atc")
            # B is stored zero-padded per expert ([K=128] with only that
            # expert's 64 r-rows non-zero) so every GEMM2 matmul of a
            # (pair, token-group) shares ONE full-K stationary: the weight
            # buffers then only ever rotate between that h-block and the
            # interleaved GEMM1 A-chunk, keeping LDWEIGHTS pipelined.
            btc = btpool.tile([128, 2, 2, O], F16, name="btc", tag="btc")
            nc.vector.memset(btc[:], 0)
            nc.scalar.dma_start(out=atc[:, 0], in_=at_d[:, 0])
            x0q = xtpool.tile([128, NG4, 4, TTS[0]], F16, name="x0q", tag="x0")
            nc.scalar.dma_start(out=x0q[:], in_=x_d[0][:])
            for s in range(2):
                nc.scalar.dma_start(
                    out=btc[64 * s : 64 * s + 64, 0, s, :],
                    in_=bt_d[64 * s : 64 * s + 64, 0, :],
                )
            nc.scalar.dma_start(out=atc[:, 1], in_=at_d[:, 1])
            for s in range(2):
                nc.scalar.dma_start(
                    out=btc[64 * s : 64 * s + 64, 1, s, :],
                    in_=bt_d[64 * s : 64 * s + 64, 1, :],
                )
            x1q = xtpool.tile([128, NG4, 4, TTS[1]], F16, name="x1q", tag="x1")
            nc.scalar.dma_start(out=x1q[:], in_=x_d[1][:])
            x2q = []
            for h in range(2):
                xc = xtpool.tile(
                    [128, NG4 // 2, 4, TTS[2]], F16, name=f"x2q{h}", tag=f"x2{h}"
                )
                nc.scalar.dma_start(out=xc[:], in_=x_d[2][h])
                x2q.append(xc)

            def g1_rhs(tnext, c):
                if tnext == 0:
                    return x0q[:, c // 4, c % 4, :]
                if tnext == 1:
                    return x1q[:, c // 4, c % 4, :]
                return x2q[c // 16][:, (c % 16) // 4, c % 4, :]

            def g1_mms(phts, tnext, c):
                rhs = g1_rhs(tnext, c)
                for p in range(2):
                    nc.tensor.matmul(
                        phts[p][:, : TTS[tnext]],
                        atc[:, p, c, :],
                        rhs,
                        start=(c == 0),
                        stop=(c == NCD - 1),
                    )

            def h_copies(phts, tnext):
                hts = []
                for p in range(2):
                    ht = htpool.tile([128, 512], F16, name=f"ht{tnext}_{p}", tag="ht")
                    if p == 0:
                        nc.vector.tensor_copy(ht[:, : TTS[tnext]], phts[p][:, : TTS[tnext]])
                    else:
                        nc.scalar.copy(ht[:, : TTS[tnext]], phts[p][:, : TTS[tnext]])
                    hts.append(ht)
                return hts

            # GEMM1 for tile 0 stands alone; later tiles interleave into
            # GEMM2. Pair 0 runs to completion first (pair-1 weights are
            # still in flight) so the first GEMM2 groups start earlier.
            phts = [
                ps_ht.tile([128, 512], FP32, name=f"pht0_{p}", tag="pht")
                for p in range(2)
            ]
            for p in range(2):
                for c in range(NCD):
                    nc.tensor.matmul(
                        phts[p][:, : TTS[0]],
                        atc[:, p, c, :],
                        g1_rhs(0, c),
                        start=(c == 0),
                        stop=(c == NCD - 1),
                    )
            hts = h_copies(phts, 0)

            tok0 = 0
            for tt in range(NT):
                ngrp = TTS[tt] // 128
                nunits = 2 * ngrp
                nxt = tt + 1 < NT
                if nxt:
                    phts_n = [
                        ps_ht.tile([128, 512], FP32, name=f"pht{tt + 1}_{p}", tag="pht")
                        for p in range(2)
                    ]
                    cpu = NCD // nunits  # G1 chunks emitted per GEMM2 group
                for p in range(2):
                    for g in range(ngrp):
                        unit = p * ngrp + g
                        ysq = [
                            yspool.tile([128, O], F16, name=f"ys{tt}_{p}_{g}_{s}", tag="ys")
                            for s in range(2)
                        ]
                        for oc2 in range(4):
                            pys = [
                                ps_y.tile([128, 1024], FP32, name=f"py{tt}_{unit}_{oc2}_{s}", tag="py")
                                for s in range(2)
                            ]
                            for half in range(2):
                                for s in range(2):
                                    col = oc2 * 1024 + half * 512
                                    nc.tensor.matmul(
                                        pys[s][:, half * 512 : half * 512 + 512],
                                        hts[p][:, g * 128 : (g + 1) * 128],
                                        btc[:, p, s, col : col + 512],
                                        start=True,
                                        stop=True,
                                    )
                            for s in range(2):
                                dst = ysq[s][:, oc2 * 1024 : (oc2 + 1) * 1024]
                                if (oc2 + s) % 2 == 0:
                                    nc.vector.tensor_copy(dst, pys[s][:])
                                else:
                                    nc.scalar.copy(dst, pys[s][:])
                        # dense G1 block for the next tile at the group
                        # boundary: keeps GEMM2's two row-strip stationaries
                        # co-resident (LDWEIGHTS stays pipelined) instead of
                        # thrashing the weight buffers every third matmul
                        if nxt:
                            for c in range(unit * cpu, (unit + 1) * cpu):
                                g1_mms(phts_n, tt + 1, c)
                        for s in range(2):
                            e = 2 * p + s
                            r0 = tok0 + g * 128
                            nc.sync.dma_start(
                                out=y_d[e, r0 : r0 + 128, :], in_=ysq[s][:]
                            )
                if nxt:
                    hts = h_copies(phts_n, tt + 1)
                tok0 += TTS[tt]
    nc.compile()
    return nc


def _get_nc():
    if "nc" not in _CACHE:
        _CACHE["nc"] = _build_nc()
    return _CACHE["nc"]


def _prep_weights(A, B):
    A = np.asarray(A, dtype=np.float32)
    B = np.asarray(B, dtype=np.float32)
    at = np.empty((128, 2, NCD, 128), dtype=np.float16)
    bt = np.empty((128, 2, O), dtype=np.float16)
    for p in range(2):
        # stationary for GEMM1: [D, 128] with expert 2p in cols 0-63, 2p+1 in 64-127
        atp = np.concatenate([A[2 * p].T, A[2 * p + 1].T], axis=1)  # [4096, 128]
        at[:, p] = atp.reshape(NCD, 128, 128).transpose(1, 0, 2)
        # moving for GEMM2: [128, O] with expert 2p rows 0-63, 2p+1 rows 64-127
        bt[:, p] = np.concatenate([B[2 * p].T, B[2 * p + 1].T], axis=0)
    return at, bt


def _pack_x(xk, n_tok):
    # [p, g4, j, t] = xk[t, (g4*4 + j)*128 + p]
    return np.ascontiguousarray(
        xk.reshape(n_tok, NG4, 4, 128).transpose(3, 1, 2, 0)
    )


def kernel(x, A, B, _trace=False):
    x = np.asarray(x, dtype=np.float32).reshape(T, D)
    at, bt = _prep_weights(A, B)

    nc = _get_nc()
    in_maps = []
    for k in range(NCORES):
        xk = x[k * TL : (k + 1) * TL].astype(np.float16)  # [TL, D]
        x2p = _pack_x(xk[TTS[0] + TTS[1] :], TTS[2])  # [128, 8, 4, 512]
        m = {
            "AT": at,
            "BT": bt,
            "x0": _pack_x(xk[: TTS[0]], TTS[0]),
            "x1": _pack_x(xk[TTS[0] : TTS[0] + TTS[1]], TTS[1]),
            "x2": np.ascontiguousarray(
                x2p.reshape(128, 2, NG4 // 2, 4, TTS[2]).transpose(1, 0, 2, 3, 4)
            ),
        }
        in_maps.append(m)
    res = run_bass_kernel_spmd(nc, in_maps, list(range(NCORES)), trace=_trace)
    if _trace:
        _CACHE["last_result"] = res

    y = np.empty((E, T, O), dtype=np.float16)
    for k in range(NCORES):
        y[:, k * TL : (k + 1) * TL, :] = res.results[k]["y"]
    return y.reshape(E, B_DIM, S, O).astype(np.float32)
